# revision 46
# baseline (speedup 1.0000x reference)
"""CosFormer attention Trainium2 kernel (8 NeuronCores, SPMD).

Math (per batch b, head h):
  Q = relu(x @ Wq), K = relu(x @ Wk), V = x @ Wv          (per-head slices)
  Qc/Qs = Q * cos(a_t)/sin(a_t); Kc/Ks likewise (a_t = t*pi/(2T))
  o[t]  = (sum_{s<=t} (Qc[t].Kc[s] + Qs[t].Ks[s]) V[s]) / z[t]
  z[t]  = max(Qc[t].zc[<=t] + Qs[t].zs[<=t], 1e-6)
  out   = (o * sigmoid(x @ Wg + bg)) @ Wo

Sharding: core c handles batch b = c // 4 and head pair hp = c % 4
(heads 2hp, 2hp+1). Each core computes a 2-head partial of the output
in token-major [T, D] fp16; the host sums 4 partials per batch in fp32.

Device algorithm: chunked scan over T in 4 "pairs" of 256 tokens
(2 subchunks of 128 = the partition dim), software-pipelined TWO pairs
deep: each iteration runs pair p's O/state/epilogue interleaved with
pair p+1's projections, elementwise front, K-transposes and scores, so
every dependent PE stage has independent filler work while ACT/DVE
produce its inputs. The out-projection is deferred one further
iteration (its matmuls fill the O-matmul wait at the next iteration's
head). The final pair drains per-subchunk so its output DMA overlaps
the remaining epilogue.

dtype: bf16 matmul operands (1 cy/row on the PE at any N; fp32 PSUM
accumulation), fp32 epilogue, fp16 output partials (partition-major
[128, subchunk, D] layout for 2KB DMA descriptor runs), summed on the
host in fp32. All inputs packed host-side into a few large DMAs,
ordered so pair-0-critical data (K|V weights, pair-0 x, gate bias,
pair-0 cos/sin rows) lands first; the 55 baseline DMA triggers (each
~600ns serialized on the Sync engine) are reduced to ~15.

PSUM budget (8 banks):
  proj ring x3  : kv(p) [K|V per subchunk], gq(p) [G si0|G si1|QT]
  attn ring x4  : ktat (K-transposes bf16 | at1 scores), at0 scores,
                  oo (o_si0|o_si1|ogT bf16), op0, op1 (out projection)
  state x1      : both heads' (KV,z) running state, cols h*66; only the
                  very first state matmul uses start=True (bank-wide
                  has_written clear), later ones accumulate/overwrite
                  their own fresh regions.
"""

import sys

sys.path.insert(0, "/opt/trn_rl_repo")

import math
from contextlib import ExitStack

import numpy as np
import ml_dtypes

import concourse.bass as bass  # noqa: F401
import concourse.tile as tile
from concourse import bacc, mybir
from concourse.bass_utils import run_bass_kernel_spmd

F32 = mybir.dt.float32
F16 = mybir.dt.float16
MM = mybir.dt.bfloat16
BF16NP = ml_dtypes.bfloat16

B, T, D = 2, 1024, 512
H, DK = 8, 64
NCORES = 8
SUB = 128          # subchunk (partition dim)
PC = 256           # pair = 2 subchunks
NPAIR = T // PC    # 4
KCH = D // 128     # 4 contraction chunks over D
SW = 66            # per-head state/V width: 64 V + z + pad

# f32 const pack columns: coss(8) | sins(8)
C_COSS, C_SINS = 0, 8
CW32 = 16
# bf16 const pack: ident | onesz | onesrow | bgrow | trig0 | mask0 | mask1 |
# trig blocks for pairs 1-3 ([cos_p(256)|sin_p(256)] each)
C_ID, C_ONE, C_ONER, C_BG = 0, 128, 130, 258
C_TRIG0 = 514
C_M0 = 1026                 # [tri|ones|tri|ones]  (512)
C_M1 = C_M0 + 512           # [tri|tri]            (256)
C_TRIG123 = C_M1 + 256
CWMM = C_TRIG123 + 3 * 512

Relu = mybir.ActivationFunctionType.Relu
Sigmoid = mybir.ActivationFunctionType.Sigmoid
Copy = mybir.ActivationFunctionType.Copy
Mult = mybir.AluOpType.mult


def _build():
    nc = bacc.Bacc("TRN2", target_bir_lowering=False, debug=False,
                   num_devices=NCORES)

    # ---- DRAM I/O (per-core data differs; program is shared) ----
    # x pair-major: xp[p, pair*1024 + k*256 + t] = x[b].T[k*128+p, pair*256+t]
    d_x = nc.dram_tensor("xp", [128, KCH * T], MM, kind="ExternalInput").ap()
    # weights: per k-chunk [K|V|G|Q] feature blocks of 128 each
    d_wall = nc.dram_tensor("wall", [128, KCH * 512], MM,
                            kind="ExternalInput").ap()
    d_wo = nc.dram_tensor("wo", [128, D], MM, kind="ExternalInput").ap()
    d_c32 = nc.dram_tensor("c32", [128, CW32], F32, kind="ExternalInput").ap()
    d_cmm = nc.dram_tensor("cmm", [128, CWMM], MM, kind="ExternalInput").ap()
    # partition-major output: y2[p, s, d] (s = subchunk); contiguous 2KB
    # per-partition runs per pair -> large DMA descriptors
    d_y = nc.dram_tensor("y", [128, (T // SUB) * D], F16,
                         kind="ExternalOutput").ap()
    d_y_r = d_y.rearrange("p (s d) -> p s d", d=D)

    with tile.TileContext(nc) as tc, ExitStack() as ctx:
        const = ctx.enter_context(tc.tile_pool(name="const", bufs=1))
        xpool = ctx.enter_context(tc.tile_pool(name="xp", bufs=1))
        work = ctx.enter_context(tc.tile_pool(name="work", bufs=2))
        ps_proj = ctx.enter_context(
            tc.tile_pool(name="psproj", bufs=3, space="PSUM"))
        ps_attn = ctx.enter_context(
            tc.tile_pool(name="psattn", bufs=4, space="PSUM"))
        ps_state = ctx.enter_context(
            tc.tile_pool(name="psstate", bufs=1, space="PSUM"))

        # ---- persistent constants, packed DMAs ----
        wall = const.tile([128, KCH * 512], MM)
        xk = xpool.tile([128, KCH * T], MM)
        c32 = const.tile([128, CW32], F32)
        cmm = const.tile([128, CWMM], MM)
        wo = const.tile([128, D], MM)

        nc.sync.dma_start(wall[:, 0:512], d_wall[:, 0:512])
        nc.sync.dma_start(xk[:, 0:2 * PC], d_x[:, 0:2 * PC])
        nc.sync.dma_start(wall[:, 512:1024], d_wall[:, 512:1024])
        nc.sync.dma_start(xk[:, 2 * PC:KCH * PC], d_x[:, 2 * PC:KCH * PC])
        nc.sync.dma_start(c32[:], d_c32[:])
        nc.sync.dma_start(cmm[:, 0:C_M0], d_cmm[:, 0:C_M0])
        nc.sync.dma_start(wall[:, 1024:], d_wall[:, 1024:])
        nc.sync.dma_start(xk[:, KCH * PC:], d_x[:, KCH * PC:])
        nc.sync.dma_start(cmm[:, C_M0:], d_cmm[:, C_M0:])
        nc.sync.dma_start(wo[:], d_wo[:])

        def xs(p, k, lo, hi):  # x slice: pair p, k-chunk k, cols [lo,hi)
            base = p * (KCH * PC) + k * PC
            return xk[:, base + lo: base + hi]

        ident = cmm[:, C_ID:C_ID + 128]
        onesz = cmm[:, C_ONE:C_ONE + 2]
        mask0 = cmm[:, C_M0:C_M0 + 512]
        mask1 = cmm[:, C_M1:C_M1 + 256]
        ones1r = cmm[0:1, C_ONER:C_ONER + 128]     # [1,128] ones
        bgrow = cmm[0:1, C_BG:C_BG + 256]          # [1,256] gate bias

        # persistent V tiles (double-buffered over pair parity), ones cols
        # written once; layout [si0 h0(66)|si0 h1(66)|si1 h0|si1 h1]
        vext = [const.tile([128, 4 * SW], MM, name=f"vext{par}")
                for par in range(2)]
        for par in range(2):
            for q in range(4):
                nc.vector.tensor_copy(
                    vext[par][:, q * SW + 64:q * SW + 66], onesz)

        state = ps_state.tile([128, 2 * SW], F32, tag="state")

        # ---- projection pieces for pair p (PE only) ----
        def kv_proj(p):
            kv = ps_proj.tile([128, 512], F32, tag="proj", name=f"kv{p}")
            for si in range(2):
                for k in range(KCH):
                    nc.tensor.matmul(kv[:, si * 256:(si + 1) * 256],
                                     xs(p, k, si * SUB, (si + 1) * SUB),
                                     wall[:, k * 256:(k + 1) * 256],
                                     start=(si == 0 and k == 0),
                                     stop=(si == 1 and k == KCH - 1),
                                     skip_group_check=True)
            return kv

        def gq_proj(p):
            gq = ps_proj.tile([128, 512], F32, tag="proj", name=f"gq{p}")
            for si in range(2):
                for k in range(KCH):
                    nc.tensor.matmul(gq[:, si * 128:(si + 1) * 128],
                                     xs(p, k, si * SUB, (si + 1) * SUB),
                                     wall[:, 1024 + k * 128:1024 + (k + 1) * 128],
                                     start=(si == 0 and k == 0), stop=False,
                                     skip_group_check=True)
            for k in range(KCH):
                nc.tensor.matmul(gq[:, 256:512],
                                 wall[:, 1536 + k * 128:1536 + (k + 1) * 128],
                                 xs(p, k, 0, PC),
                                 start=False, stop=False,
                                 skip_group_check=True)
            # gate bias: rank-1 accumulate of bg onto both G blocks
            nc.tensor.matmul(gq[:, 0:256], ones1r, bgrow,
                             start=False, stop=True, skip_group_check=True)
            return gq

        # ---- elementwise pieces (ACT / DVE) ----
        def kcat_acts(p, kv):
            kcat = [work.tile([128, 256], MM, tag=f"kcat{si}",
                              name=f"kcat{si}") for si in range(2)]
            for si in range(2):
                sub = 2 * p + si
                ksrc = kv[:, si * 256:si * 256 + 128].rearrange(
                    "p (h e) -> p h e", h=2)
                kc = kcat[si][:].rearrange("p (h c e) -> p h c e", h=2, c=2)
                nc.scalar.activation(kc[:, :, 0, :], ksrc, Relu,
                                     scale=c32[:, C_COSS + sub:C_COSS + sub + 1])
                nc.scalar.activation(kc[:, :, 1, :], ksrc, Relu,
                                     scale=c32[:, C_SINS + sub:C_SINS + sub + 1])
            return kcat

        def vext_copy(p, kv):
            for si in range(2):
                vdst = vext[p % 2][:, si * 132:(si + 1) * 132].rearrange(
                    "p (h w) -> p h w", h=2)
                vsrc = kv[:, si * 256 + 128:si * 256 + 256].rearrange(
                    "p (h e) -> p h e", h=2)
                nc.scalar.activation(vdst[:, :, 0:64], vsrc, Copy)

        def gate_act(p, gq):
            gate = work.tile([128, 256], F32, tag="gate")
            nc.scalar.activation(gate[:], gq[:, 0:256], Sigmoid)
            return gate

        def qtc_stt(p, gq):
            t0 = p * PC
            qtc = [work.tile([128, 256], MM, name=f"qtc{h}", tag=f"qtc{h}")
                   for h in range(2)]
            for h in range(2):
                hs = slice(h * 64, (h + 1) * 64)
                qsrc = gq[:, 256:512]
                tb = C_TRIG0 if p == 0 else C_TRIG123 + (p - 1) * 512
                nc.vector.scalar_tensor_tensor(
                    qtc[h][0:64, :], qsrc[hs, :], 0.0,
                    cmm[hs, tb:tb + PC],
                    op0=mybir.AluOpType.max, op1=Mult)
                nc.vector.scalar_tensor_tensor(
                    qtc[h][64:128, :], qsrc[hs, :], 0.0,
                    cmm[hs, tb + 256:tb + 512],
                    op0=mybir.AluOpType.max, op1=Mult)
            return qtc

        # ---- helper stages shared by prologue and loop ----
        def transposes(p, kcat):
            at = ps_attn.tile([128, 512], F32, tag="attn", name=f"ktat{p}")
            kt = at[:, 0:256].bitcast(MM)        # [128, 512] bf16
            for si in range(2):
                for h in range(2):
                    seg = slice((si * 2 + h) * 128, (si * 2 + h + 1) * 128)
                    nc.tensor.transpose(kt[:, seg],
                                        kcat[si][:, h * 128:(h + 1) * 128],
                                        ident)
            ktc = work.tile([128, 512], MM, tag="ktc")
            nc.vector.tensor_copy(ktc[:], kt[:])
            return at, ktc

        def scores(p, at, ktc, qtc):
            at1 = at[:, 256:512]                 # [128, 256] f32
            at0 = ps_attn.tile([128, 512], F32, tag="attn", name=f"at0{p}")
            for h in range(2):
                nc.tensor.matmul(at0[:, h * 256:(h + 1) * 256],
                                 ktc[:, h * 128:(h + 1) * 128], qtc[h][:],
                                 start=True, stop=True, skip_group_check=True)
                nc.tensor.matmul(at1[:, h * 128:(h + 1) * 128],
                                 ktc[:, 256 + h * 128:256 + (h + 1) * 128],
                                 qtc[h][:, 128:256],
                                 start=True, stop=True, skip_group_check=True)
            atm0 = work.tile([128, 512], MM, tag="atm0")
            atm1 = work.tile([128, 256], MM, tag="atm1")
            nc.vector.tensor_mul(atm0[:], at0[:], mask0)
            nc.vector.tensor_mul(atm1[:], at1[:], mask1)
            return atm0, atm1

        # ---- prologue: pair 0 through its scores ----
        kv = kv_proj(0)
        gq = gq_proj(0)
        kcat = kcat_acts(0, kv)
        vext_copy(0, kv)
        gate = gate_act(0, gq)
        qtc = qtc_stt(0, gq)
        at, ktc = transposes(0, kcat)
        atm0, atm1 = scores(0, at, ktc, qtc)
        st_sb = None
        pending = None   # (p_prev, og2t_prev): out-projection deferred one
                         # iteration so its matmuls fill the O-wait bubble

        # steady-state iteration p: O/state/epilogue of pair p interleaved
        # with pair p+1's projections, front, transposes and scores (2-deep
        # software pipeline; PE always has independent filler work)
        for p in range(NPAIR):
            first, last = (p == 0), (p == NPAIR - 1)

            # PE filler: next pair's K|V projections
            if not last:
                nkv = kv_proj(p + 1)

            # PE filler: previous pair's deferred out-projection
            if pending is not None:
                pp, og2t_prev = pending
                opps = []
                for si in range(2):
                    op_ps = ps_attn.tile([128, 512], F32, tag="attn",
                                         name=f"op{pp}{si}")
                    nc.tensor.matmul(op_ps[:],
                                     og2t_prev[:, si * 128:(si + 1) * 128],
                                     wo[:], start=True, stop=True,
                                     skip_group_check=True)
                    opps.append(op_ps)

            # ---- O[t, e] per subchunk (z in col 64 of each head slot) ----
            oo = ps_attn.tile([128, 512], F32, tag="attn", name=f"oo{p}")
            o_ns = [oo[:, 0:132], oo[:, 132:264]]
            ogt = oo[:, 264:392].bitcast(MM)     # [128, 256] bf16
            for si in range(2):
                o_ps = o_ns[si]
                for h in range(2):
                    oc = slice(h * SW, (h + 1) * SW)
                    vh0 = vext[p % 2][:, h * SW:(h + 1) * SW]
                    vh1 = vext[p % 2][:, 132 + h * SW:132 + (h + 1) * SW]
                    if si == 0:
                        nc.tensor.matmul(o_ps[:, oc],
                                         atm0[:, h * 256:h * 256 + 128], vh0,
                                         start=True, stop=first,
                                         skip_group_check=True)
                    else:
                        nc.tensor.matmul(o_ps[:, oc],
                                         atm0[:, h * 256 + 128:h * 256 + 256],
                                         vh0, start=True, stop=False,
                                         skip_group_check=True)
                        nc.tensor.matmul(o_ps[:, oc],
                                         atm1[:, h * 128:(h + 1) * 128],
                                         vh1,
                                         start=False, stop=first,
                                         skip_group_check=True)
                    if not first:
                        nc.tensor.matmul(o_ps[:, oc],
                                         qtc[h][:, si * 128:(si + 1) * 128],
                                         st_sb[:, oc],
                                         start=False, stop=True,
                                         skip_group_check=True)

            # ---- state update (PE): only the very first matmul clears ----
            for si in range(2):
                for h in range(2):
                    nc.tensor.matmul(state[:, h * SW:(h + 1) * SW],
                                     kcat[si][:, h * 128:(h + 1) * 128],
                                     vext[p % 2][:, (si * 2 + h) * SW:
                                                  (si * 2 + h + 1) * SW],
                                     start=(first and si == 0 and h == 0),
                                     stop=(last and si == 1),
                                     skip_group_check=True)

            # next pair's kcat + vext/gate up front on the scalar queue
            # (vext feeds next iteration's O matmuls -- must not queue
            # behind this pair's epilogue copies)
            if not last:
                nkcat = kcat_acts(p + 1, nkv)
                ngq = gq_proj(p + 1)

            # ---- epilogue scalars + og (DVE) ----
            zsrc = oo[:, 0:264].rearrange("p (s w) -> p s w", w=SW)[:, :, 64:65]
            rz = work.tile([128, 4], F32, tag="rz")
            nc.vector.reciprocal(rz[:], zsrc.rearrange("p s w -> p (s w)"))
            og = work.tile([128, 256], MM, tag="og")

            def og_stt(si):
                for h in range(2):
                    nc.vector.scalar_tensor_tensor(
                        og[:, si * 128 + h * 64:si * 128 + (h + 1) * 64],
                        o_ns[si][:, h * SW:h * SW + 64],
                        rz[:, 2 * si + h:2 * si + h + 1],
                        gate[:, si * 128 + h * 64:si * 128 + (h + 1) * 64],
                        op0=Mult, op1=Mult)

            if last:
                if pending is not None:
                    pp, _ = pending
                    obp = work.tile([128, 1024], F16, tag="ob")
                    for si in range(2):
                        nc.scalar.activation(obp[:, si * 512:(si + 1) * 512],
                                             opps[si][:], Copy)
                    nc.sync.dma_start(
                        d_y_r[:, 2 * pp:2 * pp + 2, :],
                        obp[:].rearrange("p (s d) -> p s d", s=2))
                # drain pair: per-subchunk chains so si0's output DMA
                # overlaps si1's epilogue
                og2t = work.tile([128, 256], MM, tag="og2t")
                ob = work.tile([128, 1024], F16, tag="ob", name="obl")
                for si in range(2):
                    og_stt(si)
                    nc.tensor.transpose(ogt[:, si * 128:(si + 1) * 128],
                                        og[:, si * 128:(si + 1) * 128], ident)
                    nc.scalar.activation(og2t[:, si * 128:(si + 1) * 128],
                                         ogt[:, si * 128:(si + 1) * 128], Copy)
                    op_ps = ps_attn.tile([128, 512], F32, tag="attn",
                                         name=f"op{p}{si}")
                    nc.tensor.matmul(op_ps[:],
                                     og2t[:, si * 128:(si + 1) * 128],
                                     wo[:], start=True, stop=True,
                                     skip_group_check=True)
                    obs = ob[:, si * 512:(si + 1) * 512]
                    nc.scalar.activation(obs[:, 0:256], op_ps[:, 0:256], Copy)
                    nc.vector.tensor_copy(obs[:, 256:512], op_ps[:, 256:512])
                    nc.sync.dma_start(d_y_r[:, 2 * p + si, 0:256],
                                      obs[:, 0:256])
                    nc.sync.dma_start(d_y_r[:, 2 * p + si, 256:512],
                                      obs[:, 256:512])
                continue

            og_stt(0)
            og_stt(1)

            # next pair's qtc (DVE) then transposes (PE) + state copy
            if not last:
                nqtc = qtc_stt(p + 1, ngq)
                nat, nktc = transposes(p + 1, nkcat)
                st_sb = work.tile([128, 2 * SW], MM, tag="stsb")
                nc.vector.tensor_copy(st_sb[:], state[:])

            # ---- gated-output transpose + out projection (PE) ----
            for si in range(2):
                nc.tensor.transpose(ogt[:, si * 128:(si + 1) * 128],
                                    og[:, si * 128:(si + 1) * 128], ident)
            og2t = work.tile([128, 256], MM, tag="og2t")
            nc.scalar.activation(og2t[:], ogt[:], Copy)

            # PE filler: next pair's scores while og2t lands
            if not last:
                natm0, natm1 = scores(p + 1, nat, nktc, nqtc)

            # previous pair's deferred output copies + DMA
            if pending is not None:
                pp, _ = pending
                obp = work.tile([128, 1024], F16, tag="ob")
                for si in range(2):
                    nc.scalar.activation(obp[:, si * 512:(si + 1) * 512],
                                         opps[si][:], Copy)
                nc.sync.dma_start(
                    d_y_r[:, 2 * pp:2 * pp + 2, :],
                    obp[:].rearrange("p (s d) -> p s d", s=2))
            pending = (p, og2t)

            # rest of next pair's front (ACT)
            if not last:
                vext_copy(p + 1, nkv)
                gate = gate_act(p + 1, ngq)
                kv, gq, kcat, qtc = nkv, ngq, nkcat, nqtc
                atm0, atm1 = natm0, natm1

    nc.finalize()
    return nc


_PROG = None


def _prog():
    global _PROG
    if _PROG is None:
        _PROG = _build()
    return _PROG


def _host_inputs(x, Wq, Wk, Wv, Wo, Wg, bg):
    x = np.asarray(x, dtype=np.float32)
    Wq = np.asarray(Wq, dtype=np.float32)
    Wk = np.asarray(Wk, dtype=np.float32)
    Wv = np.asarray(Wv, dtype=np.float32)
    Wo = np.asarray(Wo, dtype=np.float32)
    Wg = np.asarray(Wg, dtype=np.float32)
    bg = np.asarray(bg, dtype=np.float32)

    angle = np.arange(T, dtype=np.float64) * (math.pi / (2 * T))
    cosw = np.cos(angle).astype(np.float32)
    sinw = np.sin(angle).astype(np.float32)

    s = np.arange(128)[:, None]
    tl = np.arange(128)[None, :]
    tri = (s <= tl).astype(np.float32)
    ident = np.eye(128, dtype=np.float32)
    ones128 = np.ones((128, 128), dtype=np.float32)
    onesz = np.zeros((128, 2), dtype=np.float32)
    onesz[:, 0] = 1.0

    coss = np.ascontiguousarray(cosw.reshape(T // SUB, SUB).T)
    sins = np.ascontiguousarray(sinw.reshape(T // SUB, SUB).T)


    c32 = np.ascontiguousarray(
        np.concatenate([coss, sins], axis=1).astype(np.float32))

    in_maps = []
    for c in range(NCORES):
        b, hp = c // 4, c % 4
        hs = slice(hp * 128, (hp + 1) * 128)
        xT = x[b].T  # [D, T]
        xp = xT.reshape(KCH, 128, NPAIR, PC).transpose(1, 2, 0, 3) \
               .reshape(128, KCH * T)
        kvblk, gblk, qblk = [], [], []
        for k in range(KCH):
            ks = slice(k * 128, (k + 1) * 128)
            kvblk.append(np.concatenate([Wk[ks, hs], Wv[ks, hs]], axis=1))
            gblk.append(Wg[ks, hs])
            qblk.append(Wq[ks, hs])
        wall = np.concatenate(kvblk + gblk + qblk, axis=1)
        bgr = np.broadcast_to(np.concatenate([bg[hs], bg[hs]])[None, :],
                              (128, 256))
        tblk = []
        for p in range(NPAIR):
            cs = slice(p * PC, (p + 1) * PC)
            tblk += [np.broadcast_to(cosw[None, cs], (128, PC)),
                     np.broadcast_to(sinw[None, cs], (128, PC))]
        cmm = np.concatenate(
            [ident, onesz, ones128, bgr] + tblk[0:2] +
            [tri, ones128, tri, ones128,   # mask0
             tri, tri] + tblk[2:],         # mask1 | pairs 1-3 trig
            axis=1).astype(BF16NP)
        in_maps.append({
            "xp": xp.astype(BF16NP),
            "wall": wall.astype(BF16NP),
            "wo": np.ascontiguousarray(Wo[hs, :]).astype(BF16NP),
            "c32": c32,
            "cmm": np.ascontiguousarray(cmm),
        })
    return in_maps


def _install_ntff_hook():
    """The agent image's antenv lacks axon_hooks; synthesize it so
    run_bass_kernel_spmd(trace=True) can capture NTFF profiles."""
    import types
    if "antenv.axon_hooks" in sys.modules:
        return
    import antenv
    import trn_agent_boot.trn_boot as tb
    mod = types.ModuleType("antenv.axon_hooks")
    holder = [None]
    mod.set_axon_ntff_profile_hook = lambda h: holder.__setitem__(0, h)
    mod.get_axon_ntff_profile_hook = lambda: holder[0]
    sys.modules["antenv.axon_hooks"] = mod
    antenv.axon_hooks = mod
    mod.set_axon_ntff_profile_hook(
        tb._ntff_profile_via_ctypes("/opt/axon/libaxon_pjrt.so"))


def _run(inputs, trace=False):
    nc = _prog()
    if trace:
        _install_ntff_hook()
    in_maps = _host_inputs(**inputs)
    res = run_bass_kernel_spmd(nc, in_maps, core_ids=list(range(NCORES)),
                               trace=trace)
    y = np.zeros((B, T, D), dtype=np.float32)
    for c in range(NCORES):
        yc = res.results[c]["y"].astype(np.float32)
        yc = yc.reshape(128, T // SUB, D).transpose(1, 0, 2).reshape(T, D)
        y[c // 4] += yc
    return y, res


def kernel(**inputs):
    y, _ = _run(inputs, trace=False)
    return y


# revision 47
# speedup vs baseline: 1.0225x; 1.0225x over previous
"""CosFormer attention Trainium2 kernel (8 NeuronCores, SPMD).

Math (per batch b, head h):
  Q = relu(x @ Wq), K = relu(x @ Wk), V = x @ Wv          (per-head slices)
  Qc/Qs = Q * cos(a_t)/sin(a_t); Kc/Ks likewise (a_t = t*pi/(2T))
  o[t]  = (sum_{s<=t} (Qc[t].Kc[s] + Qs[t].Ks[s]) V[s]) / z[t]
  z[t]  = max(Qc[t].zc[<=t] + Qs[t].zs[<=t], 1e-6)
  out   = (o * sigmoid(x @ Wg + bg)) @ Wo

Sharding: core c handles batch b = c // 4 and head pair hp = c % 4
(heads 2hp, 2hp+1). Each core computes a 2-head partial of the output
in token-major [T, D] fp16; the host sums 4 partials per batch in fp32.

Device algorithm: chunked scan over T in 4 "pairs" of 256 tokens
(2 subchunks of 128 = the partition dim), software-pipelined TWO pairs
deep: each iteration runs pair p's O/state/epilogue interleaved with
pair p+1's projections, elementwise front, K-transposes and scores, so
every dependent PE stage has independent filler work while ACT/DVE
produce its inputs. The out-projection is deferred one further
iteration (its matmuls fill the O-matmul wait at the next iteration's
head). The final pair drains per-subchunk so its output DMA overlaps
the remaining epilogue.

dtype: bf16 matmul operands (1 cy/row on the PE at any N; fp32 PSUM
accumulation), fp32 epilogue, fp16 output partials (partition-major
[128, subchunk, D] layout for 2KB DMA descriptor runs), summed on the
host in fp32. All inputs packed host-side into a few large DMAs,
ordered so pair-0-critical data (K|V weights, pair-0 x, gate bias,
pair-0 cos/sin rows) lands first; the 55 baseline DMA triggers (each
~600ns serialized on the Sync engine) are reduced to ~15.

PSUM budget (8 banks):
  proj ring x3  : kv(p) [K|V per subchunk], gq(p) [G si0|G si1|QT]
  attn ring x4  : ktat (K-transposes bf16 | at1 scores), at0 scores,
                  oo (o_si0|o_si1|ogT bf16), op0, op1 (out projection)
  state x1      : both heads' (KV,z) running state, cols h*66; only the
                  very first state matmul uses start=True (bank-wide
                  has_written clear), later ones accumulate/overwrite
                  their own fresh regions.
"""

import sys

sys.path.insert(0, "/opt/trn_rl_repo")

import math
from contextlib import ExitStack

import numpy as np
import ml_dtypes

import concourse.bass as bass  # noqa: F401
import concourse.tile as tile
from concourse import bacc, mybir
from concourse.bass_utils import run_bass_kernel_spmd

F32 = mybir.dt.float32
F16 = mybir.dt.float16
MM = mybir.dt.bfloat16
BF16NP = ml_dtypes.bfloat16

B, T, D = 2, 1024, 512
H, DK = 8, 64
NCORES = 8
SUB = 128          # subchunk (partition dim)
PC = 256           # pair = 2 subchunks
NPAIR = T // PC    # 4
KCH = D // 128     # 4 contraction chunks over D
SW = 66            # per-head state/V width: 64 V + z + pad

# f32 const pack columns: coss(8) | sins(8)
C_COSS, C_SINS = 0, 8
CW32 = 16
# bf16 const pack: ident | onesz | onesrow | bgrow | trig0 | mask0 | mask1 |
# trig blocks for pairs 1-3 ([cos_p(256)|sin_p(256)] each)
C_ID, C_ONE, C_ONER, C_BG = 0, 128, 130, 258
C_TRIG0 = 514
C_M0 = 1026                 # [tri|ones|tri|ones]  (512)
C_M1 = C_M0 + 512           # [tri|tri]            (256)
C_TRIG123 = C_M1 + 256
CWMM = C_TRIG123 + 3 * 512

Relu = mybir.ActivationFunctionType.Relu
Sigmoid = mybir.ActivationFunctionType.Sigmoid
Copy = mybir.ActivationFunctionType.Copy
Mult = mybir.AluOpType.mult


def _build():
    nc = bacc.Bacc("TRN2", target_bir_lowering=False, debug=False,
                   num_devices=NCORES)

    # ---- DRAM I/O (per-core data differs; program is shared) ----
    # x pair-major: xp[p, pair*1024 + k*256 + t] = x[b].T[k*128+p, pair*256+t]
    d_x = nc.dram_tensor("xp", [128, KCH * T], MM, kind="ExternalInput").ap()
    # weights: per k-chunk [K|V|G|Q] feature blocks of 128 each
    d_wall = nc.dram_tensor("wall", [128, KCH * 512], MM,
                            kind="ExternalInput").ap()
    d_wo = nc.dram_tensor("wo", [128, D], MM, kind="ExternalInput").ap()
    d_c32 = nc.dram_tensor("c32", [128, CW32], F32, kind="ExternalInput").ap()
    d_cmm = nc.dram_tensor("cmm", [128, CWMM], MM, kind="ExternalInput").ap()
    # partition-major output: y2[p, s, d] (s = subchunk); contiguous 2KB
    # per-partition runs per pair -> large DMA descriptors
    d_y = nc.dram_tensor("y", [128, (T // SUB) * D], F16,
                         kind="ExternalOutput").ap()
    d_y_r = d_y.rearrange("p (s d) -> p s d", d=D)

    with tile.TileContext(nc) as tc, ExitStack() as ctx:
        const = ctx.enter_context(tc.tile_pool(name="const", bufs=1))
        xpool = ctx.enter_context(tc.tile_pool(name="xp", bufs=1))
        work = ctx.enter_context(tc.tile_pool(name="work", bufs=2))
        ps_proj = ctx.enter_context(
            tc.tile_pool(name="psproj", bufs=3, space="PSUM"))
        ps_attn = ctx.enter_context(
            tc.tile_pool(name="psattn", bufs=4, space="PSUM"))
        ps_state = ctx.enter_context(
            tc.tile_pool(name="psstate", bufs=1, space="PSUM"))

        # ---- persistent constants, packed DMAs ----
        wall = const.tile([128, KCH * 512], MM)
        xk = xpool.tile([128, KCH * T], MM)
        c32 = const.tile([128, CW32], F32)
        cmm = const.tile([128, CWMM], MM)
        wo = const.tile([128, D], MM)

        nc.sync.dma_start(wall[:, 0:512], d_wall[:, 0:512])
        nc.sync.dma_start(xk[:, 0:2 * PC], d_x[:, 0:2 * PC])
        nc.sync.dma_start(wall[:, 512:1024], d_wall[:, 512:1024])
        nc.sync.dma_start(xk[:, 2 * PC:KCH * PC], d_x[:, 2 * PC:KCH * PC])
        nc.sync.dma_start(c32[:], d_c32[:])
        nc.sync.dma_start(cmm[:, 0:C_M0], d_cmm[:, 0:C_M0])
        nc.sync.dma_start(wall[:, 1024:], d_wall[:, 1024:])
        nc.sync.dma_start(xk[:, KCH * PC:], d_x[:, KCH * PC:])
        nc.sync.dma_start(cmm[:, C_M0:], d_cmm[:, C_M0:])
        nc.sync.dma_start(wo[:], d_wo[:])

        def xs(p, k, lo, hi):  # x slice: pair p, k-chunk k, cols [lo,hi)
            base = p * (KCH * PC) + k * PC
            return xk[:, base + lo: base + hi]

        ident = cmm[:, C_ID:C_ID + 128]
        onesz = cmm[:, C_ONE:C_ONE + 2]
        mask0 = cmm[:, C_M0:C_M0 + 512]
        mask1 = cmm[:, C_M1:C_M1 + 256]
        ones1r = cmm[0:1, C_ONER:C_ONER + 128]     # [1,128] ones
        bgrow = cmm[0:1, C_BG:C_BG + 256]          # [1,256] gate bias

        # persistent V tiles (double-buffered over pair parity), ones cols
        # written once; layout [si0 h0(66)|si0 h1(66)|si1 h0|si1 h1]
        vext = [const.tile([128, 4 * SW], MM, name=f"vext{par}")
                for par in range(2)]
        for par in range(2):
            for q in range(4):
                nc.vector.tensor_copy(
                    vext[par][:, q * SW + 64:q * SW + 66], onesz)

        state = ps_state.tile([128, 2 * SW], F32, tag="state")

        # ---- projection pieces for pair p (PE only) ----
        def kv_proj(p):
            kv = ps_proj.tile([128, 512], F32, tag="proj", name=f"kv{p}")
            # k0/k1 for both subchunks first: at startup only the first
            # half of wall/x has landed, and this order doubles the work
            # available from it while k2/k3 stream in
            order = [(si, k) for kk in (0, 1) for si in (0, 1)
                     for k in (2 * kk, 2 * kk + 1)]
            for idx, (si, k) in enumerate(order):
                nc.tensor.matmul(kv[:, si * 256:(si + 1) * 256],
                                 xs(p, k, si * SUB, (si + 1) * SUB),
                                 wall[:, k * 256:(k + 1) * 256],
                                 start=(idx == 0),
                                 stop=(idx == len(order) - 1),
                                 skip_group_check=True)
            return kv

        def gq_proj(p):
            gq = ps_proj.tile([128, 512], F32, tag="proj", name=f"gq{p}")
            for si in range(2):
                for k in range(KCH):
                    nc.tensor.matmul(gq[:, si * 128:(si + 1) * 128],
                                     xs(p, k, si * SUB, (si + 1) * SUB),
                                     wall[:, 1024 + k * 128:1024 + (k + 1) * 128],
                                     start=(si == 0 and k == 0), stop=False,
                                     skip_group_check=True)
            for k in range(KCH):
                nc.tensor.matmul(gq[:, 256:512],
                                 wall[:, 1536 + k * 128:1536 + (k + 1) * 128],
                                 xs(p, k, 0, PC),
                                 start=False, stop=False,
                                 skip_group_check=True)
            # gate bias: rank-1 accumulate of bg onto both G blocks
            nc.tensor.matmul(gq[:, 0:256], ones1r, bgrow,
                             start=False, stop=True, skip_group_check=True)
            return gq

        # ---- elementwise pieces (ACT / DVE) ----
        def kcat_acts(p, kv):
            kcat = [work.tile([128, 256], MM, tag=f"kcat{si}",
                              name=f"kcat{si}") for si in range(2)]
            for si in range(2):
                sub = 2 * p + si
                ksrc = kv[:, si * 256:si * 256 + 128].rearrange(
                    "p (h e) -> p h e", h=2)
                kc = kcat[si][:].rearrange("p (h c e) -> p h c e", h=2, c=2)
                nc.scalar.activation(kc[:, :, 0, :], ksrc, Relu,
                                     scale=c32[:, C_COSS + sub:C_COSS + sub + 1])
                nc.scalar.activation(kc[:, :, 1, :], ksrc, Relu,
                                     scale=c32[:, C_SINS + sub:C_SINS + sub + 1])
            return kcat

        def vext_copy(p, kv):
            for si in range(2):
                vdst = vext[p % 2][:, si * 132:(si + 1) * 132].rearrange(
                    "p (h w) -> p h w", h=2)
                vsrc = kv[:, si * 256 + 128:si * 256 + 256].rearrange(
                    "p (h e) -> p h e", h=2)
                nc.scalar.activation(vdst[:, :, 0:64], vsrc, Copy)

        def gate_act(p, gq):
            gate = work.tile([128, 256], F32, tag="gate")
            nc.scalar.activation(gate[:], gq[:, 0:256], Sigmoid)
            return gate

        def qtc_stt(p, gq):
            t0 = p * PC
            qtc = [work.tile([128, 256], MM, name=f"qtc{h}", tag=f"qtc{h}")
                   for h in range(2)]
            for h in range(2):
                hs = slice(h * 64, (h + 1) * 64)
                qsrc = gq[:, 256:512]
                tb = C_TRIG0 if p == 0 else C_TRIG123 + (p - 1) * 512
                nc.vector.scalar_tensor_tensor(
                    qtc[h][0:64, :], qsrc[hs, :], 0.0,
                    cmm[hs, tb:tb + PC],
                    op0=mybir.AluOpType.max, op1=Mult)
                nc.vector.scalar_tensor_tensor(
                    qtc[h][64:128, :], qsrc[hs, :], 0.0,
                    cmm[hs, tb + 256:tb + 512],
                    op0=mybir.AluOpType.max, op1=Mult)
            return qtc

        # ---- helper stages shared by prologue and loop ----
        def transposes(p, kcat):
            at = ps_attn.tile([128, 512], F32, tag="attn", name=f"ktat{p}")
            kt = at[:, 0:256].bitcast(MM)        # [128, 512] bf16
            for si in range(2):
                for h in range(2):
                    seg = slice((si * 2 + h) * 128, (si * 2 + h + 1) * 128)
                    nc.tensor.transpose(kt[:, seg],
                                        kcat[si][:, h * 128:(h + 1) * 128],
                                        ident)
            ktc = work.tile([128, 512], MM, tag="ktc")
            nc.vector.tensor_copy(ktc[:], kt[:])
            return at, ktc

        def scores(p, at, ktc, qtc):
            at1 = at[:, 256:512]                 # [128, 256] f32
            at0 = ps_attn.tile([128, 512], F32, tag="attn", name=f"at0{p}")
            for h in range(2):
                nc.tensor.matmul(at0[:, h * 256:(h + 1) * 256],
                                 ktc[:, h * 128:(h + 1) * 128], qtc[h][:],
                                 start=True, stop=True, skip_group_check=True)
                nc.tensor.matmul(at1[:, h * 128:(h + 1) * 128],
                                 ktc[:, 256 + h * 128:256 + (h + 1) * 128],
                                 qtc[h][:, 128:256],
                                 start=True, stop=True, skip_group_check=True)
            atm0 = work.tile([128, 512], MM, tag="atm0")
            atm1 = work.tile([128, 256], MM, tag="atm1")
            nc.vector.tensor_mul(atm0[:], at0[:], mask0)
            nc.vector.tensor_mul(atm1[:], at1[:], mask1)
            return atm0, atm1

        # ---- prologue: pair 0 through its scores ----
        kv = kv_proj(0)
        gq = gq_proj(0)
        kcat = kcat_acts(0, kv)
        vext_copy(0, kv)
        gate = gate_act(0, gq)
        qtc = qtc_stt(0, gq)
        at, ktc = transposes(0, kcat)
        atm0, atm1 = scores(0, at, ktc, qtc)
        st_sb = None
        pending = None   # (p_prev, og2t_prev): out-projection deferred one
                         # iteration so its matmuls fill the O-wait bubble

        # steady-state iteration p: O/state/epilogue of pair p interleaved
        # with pair p+1's projections, front, transposes and scores (2-deep
        # software pipeline; PE always has independent filler work)
        for p in range(NPAIR):
            first, last = (p == 0), (p == NPAIR - 1)

            # PE filler: next pair's K|V projections
            if not last:
                nkv = kv_proj(p + 1)

            # PE filler: previous pair's deferred out-projection
            if pending is not None:
                pp, og2t_prev = pending
                opps = []
                for si in range(2):
                    op_ps = ps_attn.tile([128, 512], F32, tag="attn",
                                         name=f"op{pp}{si}")
                    nc.tensor.matmul(op_ps[:],
                                     og2t_prev[:, si * 128:(si + 1) * 128],
                                     wo[:], start=True, stop=True,
                                     skip_group_check=True)
                    opps.append(op_ps)

            # ---- O[t, e] per subchunk (z in col 64 of each head slot) ----
            oo = ps_attn.tile([128, 512], F32, tag="attn", name=f"oo{p}")
            o_ns = [oo[:, 0:132], oo[:, 132:264]]
            ogt = oo[:, 264:392].bitcast(MM)     # [128, 256] bf16
            for si in range(2):
                o_ps = o_ns[si]
                for h in range(2):
                    oc = slice(h * SW, (h + 1) * SW)
                    vh0 = vext[p % 2][:, h * SW:(h + 1) * SW]
                    vh1 = vext[p % 2][:, 132 + h * SW:132 + (h + 1) * SW]
                    if si == 0:
                        nc.tensor.matmul(o_ps[:, oc],
                                         atm0[:, h * 256:h * 256 + 128], vh0,
                                         start=True, stop=first,
                                         skip_group_check=True)
                    else:
                        nc.tensor.matmul(o_ps[:, oc],
                                         atm0[:, h * 256 + 128:h * 256 + 256],
                                         vh0, start=True, stop=False,
                                         skip_group_check=True)
                        nc.tensor.matmul(o_ps[:, oc],
                                         atm1[:, h * 128:(h + 1) * 128],
                                         vh1,
                                         start=False, stop=first,
                                         skip_group_check=True)
                    if not first:
                        nc.tensor.matmul(o_ps[:, oc],
                                         qtc[h][:, si * 128:(si + 1) * 128],
                                         st_sb[:, oc],
                                         start=False, stop=True,
                                         skip_group_check=True)

            # ---- state update (PE): only the very first matmul clears ----
            for si in range(2):
                for h in range(2):
                    nc.tensor.matmul(state[:, h * SW:(h + 1) * SW],
                                     kcat[si][:, h * 128:(h + 1) * 128],
                                     vext[p % 2][:, (si * 2 + h) * SW:
                                                  (si * 2 + h + 1) * SW],
                                     start=(first and si == 0 and h == 0),
                                     stop=(last and si == 1),
                                     skip_group_check=True)

            # next pair's kcat + vext/gate up front on the scalar queue
            # (vext feeds next iteration's O matmuls -- must not queue
            # behind this pair's epilogue copies)
            if not last:
                nkcat = kcat_acts(p + 1, nkv)
                ngq = gq_proj(p + 1)

            # ---- epilogue scalars + og (DVE) ----
            zsrc = oo[:, 0:264].rearrange("p (s w) -> p s w", w=SW)[:, :, 64:65]
            rz = work.tile([128, 4], F32, tag="rz")
            nc.vector.reciprocal(rz[:], zsrc.rearrange("p s w -> p (s w)"))
            og = work.tile([128, 256], MM, tag="og")

            def og_stt(si):
                for h in range(2):
                    nc.vector.scalar_tensor_tensor(
                        og[:, si * 128 + h * 64:si * 128 + (h + 1) * 64],
                        o_ns[si][:, h * SW:h * SW + 64],
                        rz[:, 2 * si + h:2 * si + h + 1],
                        gate[:, si * 128 + h * 64:si * 128 + (h + 1) * 64],
                        op0=Mult, op1=Mult)

            if last:
                if pending is not None:
                    pp, _ = pending
                    obp = work.tile([128, 1024], F16, tag="ob")
                    for si in range(2):
                        nc.scalar.activation(obp[:, si * 512:(si + 1) * 512],
                                             opps[si][:], Copy)
                    nc.sync.dma_start(
                        d_y_r[:, 2 * pp:2 * pp + 2, :],
                        obp[:].rearrange("p (s d) -> p s d", s=2))
                # drain pair: per-subchunk chains so si0's output DMA
                # overlaps si1's epilogue
                og2t = work.tile([128, 256], MM, tag="og2t")
                ob = work.tile([128, 1024], F16, tag="ob", name="obl")
                for si in range(2):
                    og_stt(si)
                    nc.tensor.transpose(ogt[:, si * 128:(si + 1) * 128],
                                        og[:, si * 128:(si + 1) * 128], ident)
                    nc.scalar.activation(og2t[:, si * 128:(si + 1) * 128],
                                         ogt[:, si * 128:(si + 1) * 128], Copy)
                    op_ps = ps_attn.tile([128, 512], F32, tag="attn",
                                         name=f"op{p}{si}")
                    nc.tensor.matmul(op_ps[:],
                                     og2t[:, si * 128:(si + 1) * 128],
                                     wo[:], start=True, stop=True,
                                     skip_group_check=True)
                    obs = ob[:, si * 512:(si + 1) * 512]
                    nc.scalar.activation(obs[:, 0:256], op_ps[:, 0:256], Copy)
                    nc.vector.tensor_copy(obs[:, 256:512], op_ps[:, 256:512])
                    nc.sync.dma_start(d_y_r[:, 2 * p + si, 0:256],
                                      obs[:, 0:256])
                    nc.sync.dma_start(d_y_r[:, 2 * p + si, 256:512],
                                      obs[:, 256:512])
                continue

            og_stt(0)
            og_stt(1)

            # next pair's qtc (DVE) then transposes (PE) + state copy
            if not last:
                nqtc = qtc_stt(p + 1, ngq)
                nat, nktc = transposes(p + 1, nkcat)
                st_sb = work.tile([128, 2 * SW], MM, tag="stsb")
                nc.vector.tensor_copy(st_sb[:], state[:])

            # ---- gated-output transpose + out projection (PE) ----
            for si in range(2):
                nc.tensor.transpose(ogt[:, si * 128:(si + 1) * 128],
                                    og[:, si * 128:(si + 1) * 128], ident)
            og2t = work.tile([128, 256], MM, tag="og2t")
            nc.scalar.activation(og2t[:], ogt[:], Copy)

            # PE filler: next pair's scores while og2t lands
            if not last:
                natm0, natm1 = scores(p + 1, nat, nktc, nqtc)

            # previous pair's deferred output copies + DMA
            if pending is not None:
                pp, _ = pending
                obp = work.tile([128, 1024], F16, tag="ob")
                for si in range(2):
                    nc.scalar.activation(obp[:, si * 512:(si + 1) * 512],
                                         opps[si][:], Copy)
                nc.sync.dma_start(
                    d_y_r[:, 2 * pp:2 * pp + 2, :],
                    obp[:].rearrange("p (s d) -> p s d", s=2))
            pending = (p, og2t)

            # rest of next pair's front (ACT)
            if not last:
                vext_copy(p + 1, nkv)
                gate = gate_act(p + 1, ngq)
                kv, gq, kcat, qtc = nkv, ngq, nkcat, nqtc
                atm0, atm1 = natm0, natm1

    nc.finalize()
    return nc


_PROG = None


def _prog():
    global _PROG
    if _PROG is None:
        _PROG = _build()
    return _PROG


def _host_inputs(x, Wq, Wk, Wv, Wo, Wg, bg):
    x = np.asarray(x, dtype=np.float32)
    Wq = np.asarray(Wq, dtype=np.float32)
    Wk = np.asarray(Wk, dtype=np.float32)
    Wv = np.asarray(Wv, dtype=np.float32)
    Wo = np.asarray(Wo, dtype=np.float32)
    Wg = np.asarray(Wg, dtype=np.float32)
    bg = np.asarray(bg, dtype=np.float32)

    angle = np.arange(T, dtype=np.float64) * (math.pi / (2 * T))
    cosw = np.cos(angle).astype(np.float32)
    sinw = np.sin(angle).astype(np.float32)

    s = np.arange(128)[:, None]
    tl = np.arange(128)[None, :]
    tri = (s <= tl).astype(np.float32)
    ident = np.eye(128, dtype=np.float32)
    ones128 = np.ones((128, 128), dtype=np.float32)
    onesz = np.zeros((128, 2), dtype=np.float32)
    onesz[:, 0] = 1.0

    coss = np.ascontiguousarray(cosw.reshape(T // SUB, SUB).T)
    sins = np.ascontiguousarray(sinw.reshape(T // SUB, SUB).T)


    c32 = np.ascontiguousarray(
        np.concatenate([coss, sins], axis=1).astype(np.float32))

    in_maps = []
    for c in range(NCORES):
        b, hp = c // 4, c % 4
        hs = slice(hp * 128, (hp + 1) * 128)
        xT = x[b].T  # [D, T]
        xp = xT.reshape(KCH, 128, NPAIR, PC).transpose(1, 2, 0, 3) \
               .reshape(128, KCH * T)
        kvblk, gblk, qblk = [], [], []
        for k in range(KCH):
            ks = slice(k * 128, (k + 1) * 128)
            kvblk.append(np.concatenate([Wk[ks, hs], Wv[ks, hs]], axis=1))
            gblk.append(Wg[ks, hs])
            qblk.append(Wq[ks, hs])
        wall = np.concatenate(kvblk + gblk + qblk, axis=1)
        bgr = np.broadcast_to(np.concatenate([bg[hs], bg[hs]])[None, :],
                              (128, 256))
        tblk = []
        for p in range(NPAIR):
            cs = slice(p * PC, (p + 1) * PC)
            tblk += [np.broadcast_to(cosw[None, cs], (128, PC)),
                     np.broadcast_to(sinw[None, cs], (128, PC))]
        cmm = np.concatenate(
            [ident, onesz, ones128, bgr] + tblk[0:2] +
            [tri, ones128, tri, ones128,   # mask0
             tri, tri] + tblk[2:],         # mask1 | pairs 1-3 trig
            axis=1).astype(BF16NP)
        in_maps.append({
            "xp": xp.astype(BF16NP),
            "wall": wall.astype(BF16NP),
            "wo": np.ascontiguousarray(Wo[hs, :]).astype(BF16NP),
            "c32": c32,
            "cmm": np.ascontiguousarray(cmm),
        })
    return in_maps


def _install_ntff_hook():
    """The agent image's antenv lacks axon_hooks; synthesize it so
    run_bass_kernel_spmd(trace=True) can capture NTFF profiles."""
    import types
    if "antenv.axon_hooks" in sys.modules:
        return
    import antenv
    import trn_agent_boot.trn_boot as tb
    mod = types.ModuleType("antenv.axon_hooks")
    holder = [None]
    mod.set_axon_ntff_profile_hook = lambda h: holder.__setitem__(0, h)
    mod.get_axon_ntff_profile_hook = lambda: holder[0]
    sys.modules["antenv.axon_hooks"] = mod
    antenv.axon_hooks = mod
    mod.set_axon_ntff_profile_hook(
        tb._ntff_profile_via_ctypes("/opt/axon/libaxon_pjrt.so"))


def _run(inputs, trace=False):
    nc = _prog()
    if trace:
        _install_ntff_hook()
    in_maps = _host_inputs(**inputs)
    res = run_bass_kernel_spmd(nc, in_maps, core_ids=list(range(NCORES)),
                               trace=trace)
    y = np.zeros((B, T, D), dtype=np.float32)
    for c in range(NCORES):
        yc = res.results[c]["y"].astype(np.float32)
        yc = yc.reshape(128, T // SUB, D).transpose(1, 0, 2).reshape(T, D)
        y[c // 4] += yc
    return y, res


def kernel(**inputs):
    y, _ = _run(inputs, trace=False)
    return y
